# revision 30
# baseline (speedup 1.0000x reference)
"""Spatial-transformer (tiny MLP -> affine_grid -> bilinear grid_sample)
Trainium2 Bass kernel, data-parallel over 8 NeuronCores (16 images each).

Per core:
  1. MLP on the PE: h1 = tanh(x @ W1.T + b1) via 1176 fp32 matmuls
     (contraction 128/step, PSUM-accumulated), theta = tanh(h1 @ W2.T + b2).
  2. Sampling, partitions = output rows (writeback is (c,x)-contiguous):
     - mode "id" (INDIRECT1D): per-(row-tile, i) indirect DMA, one offset
       per partition, from a 12-float/cell fp32 pair table.
     - mode "q36" (DMAGatherAnt): one gather instruction per row-tile,
       28672 int16 indices into a (y, x2)-anchored fp16 table (rows 256B
       apart, 18 fp16 of content = 3 x-units of [3ch@row y, 3ch@row y+1]),
       3-column hat-weight blend reproduces exact zero-pad bilinear.
"""
import os
import numpy as np

import concourse.bass as bass
import concourse.mybir as mybir
import concourse.tile as tile
from concourse.bass import AP
from concourse.bass_utils import run_bass_kernel_spmd

F32 = mybir.dt.float32
F16 = mybir.dt.float16
I32 = mybir.dt.int32
I16 = mybir.dt.int16
N, C, H, W = 128, 3, 224, 224
NPC = N // 8
HW = H * W
FAN1 = C * H * W
NU1 = 20
TCH = 1176  # feature f = p*1176 + t
QCH = int(os.environ.get("QCH", "147"))  # MLP streaming chunk
assert TCH % QCH == 0
JT = [(0, 128), (128, 96)]
X2 = W // 2  # 112 x2 anchors per row
CELLROW = 25088  # H * X2 cells per image
CELL_ELEMS = 128  # fp16 elems per table row (256B stride); 18 used
DEV_N = int(os.environ.get("DEV_N", str(NPC)))
QELEM = int(os.environ.get("QELEM", "18"))
# gather chunk width in x-columns; 128*GX indices per DMAGatherAnt must not
# exceed the ucode's 1024-idx ring cap
GX = int(os.environ.get("GX", "8"))
# SWDGE queues: gather chunks rotate across queues, whose transfers drain
# in parallel (measured 3.8x on the gather-bound phase)
NSQ = int(os.environ.get("NSQ", "4"))
NT = NPC * H // 128  # 28 full 128-row tiles over the core's 16*224 rows
# per-image sampling mode schedule ("id" = INDIRECT1D path (one offset per
# partition per instruction, ~1.1us each -> slow); "q" = DMAGatherAnt path
# (one 28672-idx gather per row-tile). DMAGatherAnt lives in the loadable
# `mlp` GPSIMD library — it MUST be paired with insert_library_loads() or
# the Q7 dispatcher hits an unknown opcode and wedges the device. With the
# library loaded it is both correct and ~30x cheaper in instruction count.
MODES = os.environ.get("MODES", ",".join(["q"] * 16)).split(",")
GLOBAL_TILES_DEFAULT = "0"  # global tiles only apply to the id path
# offsets per indirect-DMA instruction for the id path. HW INDIRECT1D uses
# only ONE offset per partition per instruction (extra offset columns are
# ignored and it streams contiguous runs instead) — must stay 1.
GATHER_CHUNK = int(os.environ.get("GATHER_CHUNK", "1"))
assert W % GATHER_CHUNK == 0


def _patch_tile_drain():
    """This container's walrus encodes at most ONE sem wait per CTRL
    instruction; split TileContext's exit-drain waits onto extra drains."""
    from concourse.tile import TileContext
    from concourse.vector_clock import ScopedClock

    if getattr(TileContext, "_drain_patched", False):
        return

    def _split_excess_waits(self):
        cnt = [0]
        for bb in self.nc.main_func.blocks:
            lst = bb.instructions
            i = 0
            while i < len(lst):
                ins = lst[i]
                si = ins.sync_info
                if si is not None and si.on_wait and len(si.on_wait) > 1:
                    extra = list(si.on_wait[1:])
                    keep = list(si.on_wait[:1])
                    si.on_wait.clear()
                    si.on_wait.extend(keep)
                    for w in extra:
                        cnt[0] += 1
                        nop = mybir.InstNoOp(
                            name=f"waitnop-{cnt[0]}", ins=[], outs=[]
                        )
                        nop.engine = ins.engine
                        nop.sync_info = mybir.SyncInfo(on_wait=[w], on_update=[])
                        lst.insert(i, nop)
                        i += 1
                i += 1

    def _drain_and_barrier(self, tick_clock, wait_clock):
        self._split_excess_waits()
        drain_inst = self.nc.sync.drain()
        wait_clock.add_sem_waits(
            drain_inst.ins, ScopedClock({None: tick_clock.global_clock})
        )
        si = drain_inst.ins.sync_info
        if si is not None and si.on_wait and len(si.on_wait) > 1:
            extra = list(si.on_wait[1:])
            keep = list(si.on_wait[:1])
            si.on_wait.clear()
            si.on_wait.extend(keep)
            for w in extra:
                d2 = self.nc.sync.drain(fusable=False)
                d2.ins.sync_info = mybir.SyncInfo(on_wait=[w], on_update=[])
        self.nc.all_engine_barrier()
        assert self.sems is not None
        popped = self.nc._tile_sem_poison_stack.pop()
        assert popped is self._sem_poison
        self.nc.clear_and_free_semaphores(list(self.sems.allocated().values()))
        self.nc.all_engine_barrier()

    TileContext._split_excess_waits = _split_excess_waits
    TileContext._drain_and_barrier = _drain_and_barrier
    TileContext._drain_patched = True


def _dma_gather_raw(nc, out_ap, in_ap, idxs_ap, num_idxs, elem_size, queue,
                    num_idxs_reg=None, single_packet=True):
    """InstDMAGatherAnt, non-transpose, HBM source. Mirrors bass.dma_gather
    minus the elem_size%256B assert (the ucode only requires that for
    transpose mode; non-transpose packets are ceil(elem_bytes/16KB))."""
    g = nc.gpsimd
    assert in_ap.dtype == out_ap.dtype
    elem_step = in_ap.ap[0][0]
    stride_bytes = elem_step * mybir.dt.size(in_ap.dtype)
    assert stride_bytes % 256 == 0 and stride_bytes // 256 < 256
    assert idxs_ap.dtype == I16
    _in_ap = g.lower_ap_dma(in_ap, for_custom_bir_dma=True)
    _idxs_ap = g.lower_ap(idxs_ap)
    _out_ap = g.lower_ap(out_ap)
    return g.add_instruction(
        mybir.InstDMAGatherAnt(
            name=nc.get_next_instruction_name(),
            ins=[
                *_in_ap,
                _idxs_ap,
                g.lower_val_access(
                    g.to_reg(num_idxs) if num_idxs_reg is None else num_idxs_reg
                ),
            ],
            outs=[_out_ap],
            transpose=False,
            num_idxs=num_idxs,
            elem_size=elem_size,
            stride_bytes_256=stride_bytes // 256,
            gen_mode=0,
            single_packet=single_packet,
            queue_num=queue,
            sbuf_tokens_per_rank=0,
            sbuf_free_dim_per_rank=0,
            sbuf_free_dim_pad_per_rank=0,
            sbuf_byte_offset=0,
        )
    )


def _build(nc):
    A = mybir.AluOpType
    img = nc.dram_tensor("img", [NPC, C, H, W], F32, kind="ExternalInput")
    w1 = nc.dram_tensor("W1", [NU1, FAN1], F32, kind="ExternalInput")
    b1c_d = nc.dram_tensor("b1c", [NU1, 1], F32, kind="ExternalInput")
    w2 = nc.dram_tensor("W2", [6, NU1], F32, kind="ExternalInput")
    b2c_d = nc.dram_tensor("b2c", [6, 1], F32, kind="ExternalInput")
    bias6_d = nc.dram_tensor("bias6", [6, 1], F32, kind="ExternalInput")
    ys_col_d = nc.dram_tensor("ys_col", [2, 128], F32, kind="ExternalInput")
    xs_row_d = nc.dram_tensor("xs_row", [W], F32, kind="ExternalInput")
    ys_t_d = nc.dram_tensor("ys_tile", [NT * 128], F32, kind="ExternalInput")
    sel16_d = nc.dram_tensor("sel16", [128, 16], F32, kind="ExternalInput")
    gmask_d = nc.dram_tensor("gmask", [128, 8], F32, kind="ExternalInput")
    nhw_d = nc.dram_tensor("nhw_tile", [NT * 128], F32, kind="ExternalInput")
    out = nc.dram_tensor("out", [NPC, C, H, W], F32, kind="ExternalOutput")

    theta_d = nc.dram_tensor("theta_d", [6 * NPC], F32)
    imgcat = nc.dram_tensor("imgcat", [NPC * HW + 4, 12], F32)
    imgcat2 = nc.dram_tensor("imgcat2", [NPC * CELLROW, CELL_ELEMS], F16)
    wrap_a = nc.dram_tensor("wrap_a", [2 * NPC, 128 * W], I16)
    wrap_b = nc.dram_tensor("wrap_b", [2 * NPC, 128 * W], I16)

    with tile.TileContext(nc) as tc:
        # ---------- MLP on PE ----------
        with (
            tc.tile_pool(name="mlp", bufs=2) as mp,
            tc.tile_pool(name="mpsum", bufs=1, space="PSUM") as pp,
            tc.tile_pool(name="small", bufs=1) as sp,
        ):
            h1p = pp.tile([NU1, NPC], F32, tag="h1p")
            for q4 in range(TCH // QCH):
                wq = mp.tile([128, NU1 * QCH], F32, tag="wq", name="wq")
                xq = mp.tile([128, NPC * QCH], F32, tag="xq", name="xq")
                nc.sync.dma_start(
                    out=wq[:],
                    in_=AP(w1, q4 * QCH, [[TCH, 128], [FAN1, NU1], [1, QCH]]),
                )
                nc.sync.dma_start(
                    out=xq[:],
                    in_=AP(img, q4 * QCH, [[TCH, 128], [FAN1, NPC], [1, QCH]]),
                )
                for tl in range(QCH):
                    k = q4 * QCH + tl
                    nc.tensor.matmul(
                        out=h1p[:],
                        lhsT=wq[:, tl :: QCH],
                        rhs=xq[:, tl :: QCH],
                        start=(k == 0),
                        stop=(k == TCH - 1),
                    )
            b1c = sp.tile([NU1, 1], F32, tag="b1c")
            nc.sync.dma_start(out=b1c[:], in_=b1c_d[:])
            h1r = sp.tile([NU1, NPC], F32, tag="h1r")
            nc.vector.tensor_scalar(
                out=h1r[:], in0=h1p[:], scalar1=b1c[:], scalar2=None, op0=A.add
            )
            nc.scalar.activation(
                out=h1r[:], in_=h1r[:], func=mybir.ActivationFunctionType.Tanh
            )
            w2t = sp.tile([NU1, 6], F32, tag="w2t")
            nc.sync.dma_start(out=w2t[:], in_=AP(w2, 0, [[1, NU1], [NU1, 6]]))
            thp = pp.tile([6, NPC], F32, tag="thp")
            nc.tensor.matmul(
                out=thp[:], lhsT=w2t[:], rhs=h1r[:], start=True, stop=True
            )
            b2c = sp.tile([6, 1], F32, tag="b2c")
            nc.sync.dma_start(out=b2c[:], in_=b2c_d[:])
            ths = sp.tile([6, NPC], F32, tag="ths")
            nc.vector.tensor_scalar(
                out=ths[:], in0=thp[:], scalar1=b2c[:], scalar2=None, op0=A.add
            )
            nc.scalar.activation(
                out=ths[:], in_=ths[:], func=mybir.ActivationFunctionType.Tanh
            )
            bias6 = sp.tile([6, 1], F32, tag="bias6")
            nc.sync.dma_start(out=bias6[:], in_=bias6_d[:])
            nc.vector.tensor_scalar(
                out=ths[:], in0=ths[:], scalar1=112.0, scalar2=bias6[:],
                op0=A.mult, op1=A.add,
            )
            nc.sync.dma_start(
                out=AP(theta_d, 0, [[NPC, 6], [1, NPC]]), in_=ths[:]
            )

        # ---------- sampling ----------
        with tc.tile_pool(name="smp", bufs=1) as smp:
            thb = smp.tile([128, 6 * NPC], F32, tag="thb")
            nc.sync.dma_start(
                out=thb[:], in_=AP(theta_d, 0, [[0, 128], [1, 6 * NPC]])
            )
            xsr = smp.tile([128, W], F32, tag="xsr")
            nc.sync.dma_start(out=xsr[:], in_=AP(xs_row_d, 0, [[0, 128], [1, W]]))
            ycol = []
            for ti in range(2):
                yt = smp.tile([128, 1], F32, tag=f"ycol{ti}", name=f"ycol{ti}")
                nc.sync.dma_start(
                    out=yt[:], in_=AP(ys_col_d, ti * 128, [[1, 128], [0, 1]])
                )
                ycol.append(yt)
            sel16 = smp.tile([128, 16], F32, tag="sel16")
            nc.sync.dma_start(out=sel16[:], in_=sel16_d[:])
            gmask = smp.tile([128, 8], F32, tag="gmask")
            nc.sync.dma_start(out=gmask[:], in_=gmask_d[:])

            with (
                tc.tile_pool(name="icat", bufs=2) as ip,
                tc.tile_pool(name="coord", bufs=int(os.environ.get("CPB", "3"))) as cp,
                tc.tile_pool(name="quad", bufs=int(os.environ.get("QPB", "3"))) as qp,
                tc.tile_pool(name="wpsum", bufs=2, space="PSUM") as wp,
            ):
                if (os.environ.get("GLOBAL_TILES", GLOBAL_TILES_DEFAULT) == "1"
                        and DEV_N == NPC):
                    # 28 full 128-row tiles over all 16 images (no 96-row
                    # waste): 6272 gathers instead of 7168
                    for n in range(NPC):
                        _build_imgcat(nc, ip, img, imgcat, n)
                    for t in range(NT):
                        _sample_tile_g(
                            nc, cp, qp, imgcat, out, theta_d, ys_t_d, nhw_d,
                            xsr, t,
                        )
                else:
                    nreg = {
                        xn: nc.gpsimd.to_reg(128 * xn)
                        for xn in {GX, W % GX} if xn
                    }
                    for n in range(DEV_N):
                        mode = MODES[n]
                        if mode == "id":
                            _build_imgcat(nc, ip, img, imgcat, n)
                            for ti, (r0, rn) in enumerate(JT):
                                _sample_tile_id(
                                    nc, cp, qp, imgcat, out, thb, ycol[ti],
                                    xsr, n, r0, rn,
                                )
                        else:
                            _build_imgcat2(nc, ip, img, imgcat2, n)
                            for ti, (r0, rn) in enumerate(JT):
                                _sample_tile_q(
                                    nc, cp, qp, wp, sel16, gmask, imgcat2, out,
                                    thb, ycol[ti], xsr, n, ti, r0, rn,
                                    queue=(2 * n + ti) % NSQ,
                                    nreg=nreg,
                                )
    return nc


def _build_imgcat(nc, ip, img, imgcat, n):
    """fp32 pair table: row r = n*HW + y*W + x holds
    [c0..2@(y,x), c0..2@(y,x+1), c0..2@(y+1,x), c0..2@(y+1,x+1)] (clamped)."""
    for (y0, yn) in ((0, 128), (128, 96)):
        src = ip.tile([128, C * W], F32, tag="csrc", name="csrc")
        nc.sync.dma_start(
            out=src[:yn, :],
            in_=AP(img, n * FAN1 + y0 * W, [[W, yn], [HW, C], [1, W]]),
        )
        nxt = ip.tile([128, C * W], F32, tag="cnxt", name="cnxt")
        y1 = min(y0 + 1, H - 1)
        n1 = min(yn, H - y1)
        nc.sync.dma_start(
            out=nxt[:n1, :],
            in_=AP(img, n * FAN1 + y1 * W, [[W, n1], [HW, C], [1, W]]),
        )
        if n1 < yn:
            nc.sync.dma_start(out=nxt[n1:yn, :], in_=src[yn - 1 : yn, :])
        cat = ip.tile([128, W * 12], F32, tag="cat", name="cat")
        for c in range(C):
            for slot, tsrc, xoff in (
                (c, src, 0), (3 + c, src, 1), (6 + c, nxt, 0), (9 + c, nxt, 1),
            ):
                nc.scalar.copy(
                    out=cat[:yn, slot :: 12][:, 0 : W - xoff],
                    in_=tsrc[:yn, c * W + xoff : c * W + W],
                )
                if xoff:
                    nc.scalar.copy(
                        out=cat[:yn, (W - 1) * 12 + slot : (W - 1) * 12 + slot + 1],
                        in_=tsrc[:yn, c * W + W - 1 : c * W + W],
                    )
        nc.sync.dma_start(
            out=AP(imgcat, (n * HW + y0 * W) * 12, [[W * 12, yn], [1, W * 12]]),
            in_=cat[:yn, :],
        )


def _build_imgcat2(nc, ip, img, imgcat2, n):
    """fp16 hat table: row (y, x2) holds 3 x-units for x = 2*x2 + dx,
    unit = [c0,c1,c2 @ row y, c0,c1,c2 @ row y+1]; 18 fp16 at 256B stride.
    Rows exist for y in [0,223]; only y<=222 are ever gathered."""
    for (y0, yn) in ((0, 128), (128, 96)):
        src = ip.tile([128, C * W], F32, tag="csrc", name="csrc")
        nc.sync.dma_start(
            out=src[:yn, :],
            in_=AP(img, n * FAN1 + y0 * W, [[W, yn], [HW, C], [1, W]]),
        )
        nxt = ip.tile([128, C * W], F32, tag="cnxt", name="cnxt")
        y1 = min(y0 + 1, H - 1)
        n1 = min(yn, H - y1)
        nc.sync.dma_start(
            out=nxt[:n1, :],
            in_=AP(img, n * FAN1 + y1 * W, [[W, n1], [HW, C], [1, W]]),
        )
        if n1 < yn:
            nc.sync.dma_start(out=nxt[n1:yn, :], in_=src[yn - 1 : yn, :])
        cat2 = ip.tile([128, X2 * 18], F16, tag="cat2", name="cat2")
        # the dx=2 slots of x2=111 are never filled (x=224 OOB); zero the
        # tile first so masked-out hat taps multiply 0, not DRAM garbage/NaN
        nc.vector.memset(cat2[:yn, :], 0.0)
        for dx in range(3):
            # x = 2*x2 + dx <= 223  ->  x2 <= (223-dx)/2
            nx2 = min(X2, (223 - dx) // 2 + 1)
            for r, tsrc in ((0, src), (1, nxt)):
                for c in range(C):
                    s = dx * 6 + r * 3 + c
                    nc.scalar.copy(
                        out=cat2[:yn, s :: 18][:, 0:nx2],
                        in_=tsrc[:yn, c * W + dx : c * W + dx + 2 * nx2 - 1 : 2],
                    )
        nc.sync.dma_start(
            out=AP(
                imgcat2,
                (n * CELLROW + y0 * X2) * CELL_ELEMS,
                [[X2 * CELL_ELEMS, yn], [CELL_ELEMS, X2], [1, 18]],
            ),
            in_=cat2[:yn, :],
        )


def _coords(nc, cp, thb, ycol, xsr, n):
    """ix, iy [128, W] in pixel space for this row-tile."""
    A = mybir.AluOpType
    th = lambda k: thb[:, k * NPC + n : k * NPC + n + 1]
    cx = cp.tile([128, 1], F32, tag="cx", name="cx")
    cy = cp.tile([128, 1], F32, tag="cy", name="cy")
    nc.vector.tensor_scalar(
        out=cx[:], in0=ycol[:], scalar1=th(1), scalar2=th(2), op0=A.mult, op1=A.add
    )
    nc.vector.tensor_scalar(
        out=cy[:], in0=ycol[:], scalar1=th(4), scalar2=th(5), op0=A.mult, op1=A.add
    )
    ix = cp.tile([128, W], F32, tag="ix", name="ix")
    iy = cp.tile([128, W], F32, tag="iy", name="iy")
    nc.vector.tensor_scalar(
        out=ix[:], in0=xsr[:], scalar1=th(0), scalar2=cx[:], op0=A.mult, op1=A.add
    )
    nc.vector.tensor_scalar(
        out=iy[:], in0=xsr[:], scalar1=th(3), scalar2=cy[:], op0=A.mult, op1=A.add
    )
    return ix, iy


def _sample_tile_g(nc, cp, qp, imgcat, out, theta_d, ys_t_d, nhw_d, xsr, t):
    """Global row-tile t covers rows [t*128, (t+1)*128) of the core's
    16*224-row space; may span two images. Per-partition theta/ys/n*HW
    columns are assembled from DRAM (<=2 broadcast runs per column)."""
    A = mybir.AluOpType
    g0 = t * 128
    n0, r0 = g0 // H, g0 % H
    s = min(128, H - r0)  # rows from image n0; rest from n0+1
    tht = cp.tile([128, 6], F32, tag="tht", name="tht")
    for k in range(6):
        nc.sync.dma_start(
            out=tht[0:s, k : k + 1],
            in_=AP(theta_d, k * NPC + n0, [[0, s], [0, 1]]),
        )
        if s < 128:
            nc.sync.dma_start(
                out=tht[s:128, k : k + 1],
                in_=AP(theta_d, k * NPC + n0 + 1, [[0, 128 - s], [0, 1]]),
            )
    ycol = cp.tile([128, 1], F32, tag="ycolg", name="ycolg")
    nc.sync.dma_start(out=ycol[:], in_=AP(ys_t_d, g0, [[1, 128], [0, 1]]))
    nhw = cp.tile([128, 1], F32, tag="nhw", name="nhw")
    nc.sync.dma_start(out=nhw[:], in_=AP(nhw_d, g0, [[1, 128], [0, 1]]))

    t_ = lambda tag: cp.tile([128, W], F32, tag=tag, name=tag)
    th = lambda k: tht[:, k : k + 1]
    cx = cp.tile([128, 1], F32, tag="cx", name="cx")
    cy = cp.tile([128, 1], F32, tag="cy", name="cy")
    nc.vector.tensor_scalar(
        out=cx[:], in0=ycol[:], scalar1=th(1), scalar2=th(2), op0=A.mult, op1=A.add
    )
    nc.vector.tensor_scalar(
        out=cy[:], in0=ycol[:], scalar1=th(4), scalar2=th(5), op0=A.mult, op1=A.add
    )
    ix, iy = t_("ix"), t_("iy")
    nc.vector.tensor_scalar(
        out=ix[:], in0=xsr[:], scalar1=th(0), scalar2=cx[:], op0=A.mult, op1=A.add
    )
    nc.vector.tensor_scalar(
        out=iy[:], in0=xsr[:], scalar1=th(3), scalar2=cy[:], op0=A.mult, op1=A.add
    )
    fx, fy, x0, y0 = t_("fx"), t_("fy"), t_("x0"), t_("y0")
    icast = cp.tile([128, W], I32, tag="icast", name="icast")
    for v0, iv, fv in ((x0, ix, fx), (y0, iy, fy)):
        nc.vector.tensor_scalar(
            out=icast[:], in0=iv[:], scalar1=0.5, scalar2=None, op0=A.subtract
        )
        nc.vector.tensor_copy(out=v0[:], in_=icast[:])
        nc.vector.tensor_tensor(out=fv[:], in0=iv[:], in1=v0[:], op=A.subtract)

    wL, wR, wT, wB_ = t_("wL"), t_("wR"), t_("wT"), t_("wB")
    m1, m2, m3, tmp = t_("m1"), t_("m2"), t_("m3"), t_("tmp")

    def edge_w(v0, fv, hi, wA, wB):
        nc.vector.tensor_scalar(out=m1[:], in0=v0[:], scalar1=0.0, scalar2=None, op0=A.is_ge)
        nc.vector.tensor_scalar(out=m2[:], in0=v0[:], scalar1=float(hi), scalar2=None, op0=A.is_le)
        nc.vector.tensor_tensor(out=m1[:], in0=m1[:], in1=m2[:], op=A.mult)
        nc.vector.tensor_scalar(out=m2[:], in0=v0[:], scalar1=-1.0, scalar2=None, op0=A.is_equal)
        nc.vector.tensor_scalar(out=m3[:], in0=v0[:], scalar1=float(hi + 1), scalar2=None, op0=A.is_equal)
        nc.vector.tensor_tensor(out=tmp[:], in0=m2[:], in1=m1[:], op=A.subtract)
        nc.vector.tensor_tensor(out=tmp[:], in0=tmp[:], in1=fv[:], op=A.mult)
        nc.vector.tensor_tensor(out=wA[:], in0=m1[:], in1=tmp[:], op=A.add)
        nc.vector.tensor_tensor(out=tmp[:], in0=m1[:], in1=m3[:], op=A.subtract)
        nc.vector.tensor_tensor(out=tmp[:], in0=tmp[:], in1=fv[:], op=A.mult)
        nc.vector.tensor_tensor(out=wB[:], in0=m3[:], in1=tmp[:], op=A.add)

    edge_w(x0, fx, W - 2, wL, wR)
    edge_w(y0, fy, H - 2, wT, wB_)
    wTL, wTR, wBL, wBR = t_("wTL"), t_("wTR"), t_("wBL"), t_("wBR")
    nc.vector.tensor_tensor(out=wTL[:], in0=wT[:], in1=wL[:], op=A.mult)
    nc.vector.tensor_tensor(out=wTR[:], in0=wT[:], in1=wR[:], op=A.mult)
    nc.vector.tensor_tensor(out=wBL[:], in0=wB_[:], in1=wL[:], op=A.mult)
    nc.vector.tensor_tensor(out=wBR[:], in0=wB_[:], in1=wR[:], op=A.mult)
    nc.vector.tensor_scalar(out=m1[:], in0=y0[:], scalar1=0.0, scalar2=float(H - 1), op0=A.max, op1=A.min)
    nc.vector.tensor_scalar(out=m2[:], in0=x0[:], scalar1=0.0, scalar2=float(W - 2), op0=A.max, op1=A.min)
    nc.vector.tensor_scalar(out=m1[:], in0=m1[:], scalar1=float(W), scalar2=nhw[:], op0=A.mult, op1=A.add)
    nc.vector.tensor_tensor(out=m1[:], in0=m1[:], in1=m2[:], op=A.add)
    offs = cp.tile([128, W], I32, tag="offs", name="offs")
    nc.vector.tensor_copy(out=offs[:], in_=m1[:])

    quad = qp.tile([128, W * 12], F32, tag="quad", name="quad")
    gc = GATHER_CHUNK
    for i in range(0, W, gc):
        nc.gpsimd.indirect_dma_start(
            out=quad[:, i * 12 : (i + gc) * 12],
            out_offset=None,
            in_=imgcat[:],
            in_offset=bass.IndirectOffsetOnAxis(ap=offs[:, i : i + gc], axis=0),
        )
    res = qp.tile([128, C * W], F32, tag="res", name="res")
    acc1, acc2 = t_("acc1"), t_("acc2")
    for c in range(C):
        nc.vector.tensor_tensor(out=acc1[:], in0=quad[:, c :: 12], in1=wTL[:], op=A.mult)
        nc.vector.tensor_tensor(out=acc2[:], in0=quad[:, 3 + c :: 12], in1=wTR[:], op=A.mult)
        nc.vector.tensor_tensor(out=acc1[:], in0=acc1[:], in1=acc2[:], op=A.add)
        nc.vector.tensor_tensor(out=acc2[:], in0=quad[:, 6 + c :: 12], in1=wBL[:], op=A.mult)
        nc.vector.tensor_tensor(out=acc1[:], in0=acc1[:], in1=acc2[:], op=A.add)
        nc.vector.tensor_tensor(out=acc2[:], in0=quad[:, 9 + c :: 12], in1=wBR[:], op=A.mult)
        nc.vector.tensor_tensor(out=res[:, c * W : (c + 1) * W], in0=acc1[:], in1=acc2[:], op=A.add)
    nc.sync.dma_start(
        out=AP(out, n0 * FAN1 + r0 * W, [[W, s], [HW, C], [1, W]]),
        in_=res[0:s, :],
    )
    if s < 128:
        nc.sync.dma_start(
            out=AP(out, (n0 + 1) * FAN1, [[W, 128 - s], [HW, C], [1, W]]),
            in_=res[s:128, :],
        )


def _sample_tile_id(nc, cp, qp, imgcat, out, thb, ycol, xsr, n, r0, rn):
    """Proven INDIRECT1D path (one offset per partition per instruction)."""
    A = mybir.AluOpType
    t = lambda tag: cp.tile([128, W], F32, tag=tag, name=tag)
    ix, iy = _coords(nc, cp, thb, ycol, xsr, n)
    fx, fy, x0, y0 = t("fx"), t("fy"), t("x0"), t("y0")
    icast = cp.tile([128, W], I32, tag="icast", name="icast")
    for v0, iv, fv in ((x0, ix, fx), (y0, iy, fy)):
        nc.vector.tensor_scalar(
            out=icast[:], in0=iv[:], scalar1=0.5, scalar2=None, op0=A.subtract
        )
        nc.vector.tensor_copy(out=v0[:], in_=icast[:])
        nc.vector.tensor_tensor(out=fv[:], in0=iv[:], in1=v0[:], op=A.subtract)

    wL, wR, wT, wB_ = t("wL"), t("wR"), t("wT"), t("wB")
    m1, m2, m3, tmp = t("m1"), t("m2"), t("m3"), t("tmp")

    def edge_w(v0, fv, hi, wA, wB):
        nc.vector.tensor_scalar(out=m1[:], in0=v0[:], scalar1=0.0, scalar2=None, op0=A.is_ge)
        nc.vector.tensor_scalar(out=m2[:], in0=v0[:], scalar1=float(hi), scalar2=None, op0=A.is_le)
        nc.vector.tensor_tensor(out=m1[:], in0=m1[:], in1=m2[:], op=A.mult)
        nc.vector.tensor_scalar(out=m2[:], in0=v0[:], scalar1=-1.0, scalar2=None, op0=A.is_equal)
        nc.vector.tensor_scalar(out=m3[:], in0=v0[:], scalar1=float(hi + 1), scalar2=None, op0=A.is_equal)
        nc.vector.tensor_tensor(out=tmp[:], in0=m2[:], in1=m1[:], op=A.subtract)
        nc.vector.tensor_tensor(out=tmp[:], in0=tmp[:], in1=fv[:], op=A.mult)
        nc.vector.tensor_tensor(out=wA[:], in0=m1[:], in1=tmp[:], op=A.add)
        nc.vector.tensor_tensor(out=tmp[:], in0=m1[:], in1=m3[:], op=A.subtract)
        nc.vector.tensor_tensor(out=tmp[:], in0=tmp[:], in1=fv[:], op=A.mult)
        nc.vector.tensor_tensor(out=wB[:], in0=m3[:], in1=tmp[:], op=A.add)

    edge_w(x0, fx, W - 2, wL, wR)
    edge_w(y0, fy, H - 2, wT, wB_)
    wTL, wTR, wBL, wBR = t("wTL"), t("wTR"), t("wBL"), t("wBR")
    nc.vector.tensor_tensor(out=wTL[:], in0=wT[:], in1=wL[:], op=A.mult)
    nc.vector.tensor_tensor(out=wTR[:], in0=wT[:], in1=wR[:], op=A.mult)
    nc.vector.tensor_tensor(out=wBL[:], in0=wB_[:], in1=wL[:], op=A.mult)
    nc.vector.tensor_tensor(out=wBR[:], in0=wB_[:], in1=wR[:], op=A.mult)
    nc.vector.tensor_scalar(out=m1[:], in0=y0[:], scalar1=0.0, scalar2=float(H - 1), op0=A.max, op1=A.min)
    nc.vector.tensor_scalar(out=m2[:], in0=x0[:], scalar1=0.0, scalar2=float(W - 2), op0=A.max, op1=A.min)
    nc.vector.tensor_scalar(out=m1[:], in0=m1[:], scalar1=float(W), scalar2=float(n * HW), op0=A.mult, op1=A.add)
    nc.vector.tensor_tensor(out=m1[:], in0=m1[:], in1=m2[:], op=A.add)
    offs = cp.tile([128, W], I32, tag="offs", name="offs")
    nc.vector.tensor_copy(out=offs[:], in_=m1[:])

    quad = qp.tile([128, W * 12], F32, tag="quad", name="quad")
    for i in range(W):
        nc.gpsimd.indirect_dma_start(
            out=quad[:, i * 12 : (i + 1) * 12],
            out_offset=None,
            in_=imgcat[:],
            in_offset=bass.IndirectOffsetOnAxis(ap=offs[:, i : i + 1], axis=0),
        )
    res = qp.tile([128, C * W], F32, tag="res", name="res")
    acc1, acc2 = t("acc1"), t("acc2")
    for c in range(C):
        nc.vector.tensor_tensor(out=acc1[:], in0=quad[:, c :: 12], in1=wTL[:], op=A.mult)
        nc.vector.tensor_tensor(out=acc2[:], in0=quad[:, 3 + c :: 12], in1=wTR[:], op=A.mult)
        nc.vector.tensor_tensor(out=acc1[:], in0=acc1[:], in1=acc2[:], op=A.add)
        nc.vector.tensor_tensor(out=acc2[:], in0=quad[:, 6 + c :: 12], in1=wBL[:], op=A.mult)
        nc.vector.tensor_tensor(out=acc1[:], in0=acc1[:], in1=acc2[:], op=A.add)
        nc.vector.tensor_tensor(out=acc2[:], in0=quad[:, 9 + c :: 12], in1=wBR[:], op=A.mult)
        nc.vector.tensor_tensor(out=res[:, c * W : (c + 1) * W], in0=acc1[:], in1=acc2[:], op=A.add)
    nc.sync.dma_start(
        out=AP(out, n * FAN1 + r0 * W, [[W, rn], [HW, C], [1, W]]),
        in_=res[:rn, :],
    )


def _sample_tile_q(
    nc, cp, qp, wp, sel16, gmask, imgcat2, out, thb, ycol, xsr, n, ti, r0, rn,
    queue, nreg,
):
    """DMAGatherAnt path: 28 chunked 1024-idx gathers, 3-col hat blend."""
    A = mybir.AluOpType
    t = lambda tag: cp.tile([128, W], F32, tag=tag, name=tag)
    ti32 = lambda tag: cp.tile([128, W], I32, tag=tag, name=tag)
    ix, iy = _coords(nc, cp, thb, ycol, xsr, n)

    # floor -> clamp -> x2 anchor / cell offsets, all in fp32 (values exact)
    icast = cp.tile([128, W], I32, tag="icastq", name="icastq")
    tmp = t("tmpq")
    x0f, y0f = t("x0f"), t("y0f")
    for iv, vf in ((ix, x0f), (iy, y0f)):
        nc.vector.tensor_scalar(out=tmp[:], in0=iv[:], scalar1=0.5, scalar2=None, op0=A.subtract)
        nc.vector.tensor_copy(out=icast[:], in_=tmp[:])
        nc.vector.tensor_copy(out=vf[:], in_=icast[:])
    nc.vector.tensor_scalar(out=x0f[:], in0=x0f[:], scalar1=0.0, scalar2=float(W - 2), op0=A.max, op1=A.min)
    nc.vector.tensor_scalar(out=y0f[:], in0=y0f[:], scalar1=0.0, scalar2=float(H - 2), op0=A.max, op1=A.min)
    # x2 = floor(x0c/2) via round-to-nearest of x0c/2 - 0.25 (exact quarters)
    x2f = t("x2f")
    nc.vector.tensor_scalar(out=tmp[:], in0=x0f[:], scalar1=0.5, scalar2=-0.25, op0=A.mult, op1=A.add)
    nc.vector.tensor_copy(out=icast[:], in_=tmp[:])
    nc.vector.tensor_copy(out=x2f[:], in_=icast[:])
    # cell index = y0c*112 + x2 (kept in f32; values are exact integers)
    offs_f = t("offs_f")
    nc.vector.tensor_scalar(out=offs_f[:], in0=y0f[:], scalar1=float(X2), scalar2=None, op0=A.mult)
    nc.vector.tensor_tensor(out=offs_f[:], in0=offs_f[:], in1=x2f[:], op=A.add)

    dxv, dyv = t("dxv"), t("dyv")
    nc.vector.tensor_scalar(out=dxv[:], in0=x2f[:], scalar1=-2.0, scalar2=None, op0=A.mult)
    nc.vector.tensor_tensor(out=dxv[:], in0=dxv[:], in1=ix[:], op=A.add)
    nc.vector.tensor_tensor(out=dyv[:], in0=iy[:], in1=y0f[:], op=A.subtract)

    # hat weights on the Act engine: wx_k = relu(1 - |dxv - k|), k = 0,1,2
    # (k=2 masked at the right edge); wy_r likewise for r = 0,1
    AF = mybir.ActivationFunctionType
    tmph = t("tmph")
    tmph2 = t("tmph2")

    def hat(dst, dv, k):
        src = dv
        if k:
            nc.vector.tensor_scalar(
                out=tmph2[:], in0=dv[:], scalar1=float(k), scalar2=None,
                op0=A.subtract,
            )
            src = tmph2
        nc.scalar.activation(out=tmph[:], in_=src[:], func=AF.Abs)
        nc.scalar.activation(out=dst[:], in_=tmph[:], func=AF.Relu, bias=1.0, scale=-1.0)

    wx0, wx1, wx2 = t("wx0"), t("wx1"), t("wx2")
    wy0, wy1 = t("wy0"), t("wy1")
    hat(wx0, dxv, 0)
    hat(wx1, dxv, 1)
    hat(wx2, dxv, 2)
    msk = t("msk")
    nc.vector.tensor_scalar(out=msk[:], in0=x2f[:], scalar1=110.5, scalar2=None, op0=A.is_le)
    nc.vector.tensor_tensor(out=wx2[:], in0=wx2[:], in1=msk[:], op=A.mult)
    hat(wy0, dyv, 0)
    hat(wy1, dyv, 1)
    wc = {}
    for dx, wx in ((0, wx0), (1, wx1), (2, wx2)):
        for r, wy in ((0, wy0), (1, wy1)):
            w = cp.tile([128, W], F32, tag=f"wc{dx}{r}", name=f"wc{dx}{r}")
            nc.vector.tensor_tensor(out=w[:], in0=wx[:], in1=wy[:], op=A.mult)
            wc[(dx, r)] = w

    # idx wrap bounce: SBUF [128, 224] i16 -> DRAM 16-partition wrap layout
    # idx wrap for the gather ucode: index i = x*128 + p must sit at
    # partition i%16, column i//16 = x*8 + p//16, replicated to all 8
    # 16-partition groups (one per Q7 core). A direct DMA of this
    # interleave degenerates to 28672 2-byte descriptors (~84us/tile), so
    # build it on-chip instead:
    #   1. scatter offs_f into a zeroed [128, 8W] tile: partition 16u+q
    #      writes its row at columns u::8 (8 strided Act copies)
    #   2. PE one-hot matmul sums the 16-partition groups -> [16, 8W]
    #      (disjoint supports, so the sum is a select)
    #   3. cast f32 -> i16, replicate to 8 groups via SBUF->SBUF DMA
    # contiguous u-blocks (strided DVE writes are ~10x slower than packed)
    idxs8 = cp.tile([128, 8 * W], F32, tag="idxs8", name="idxs8")
    for u in range(8):
        # block u <- offs_f * [p//16 == u]; engines require full-128
        # partition APs, so mask instead of slicing partitions
        nc.vector.tensor_scalar(
            out=idxs8[:, u * W : (u + 1) * W], in0=offs_f[:],
            scalar1=gmask[:, u : u + 1], scalar2=None, op0=A.mult,
        )
    # one matmul per u-block into a 256-col PSUM slot (none crosses the
    # 2KB bank boundary); the final cast does the (k*8+u) interleave
    pw = wp.tile([16, 2048], F32, tag="pwrap")
    for u in range(8):
        nc.tensor.matmul(
            out=pw[:, u * 256 : u * 256 + W],
            lhsT=sel16[:],
            rhs=idxs8[:, u * W : (u + 1) * W],
            start=True,
            stop=True,
        )
    idx16s = cp.tile([16, 8 * W], I16, tag="idx16s", name="idx16s")
    nc.scalar.copy(
        out=idx16s[:].rearrange("p (k u) -> p k u", u=8),
        in_=pw[:].rearrange("p (u k) -> p k u", k=256)[:, 0:W, :],
    )
    idx16w = cp.tile([128, 8 * W], I16, tag="idx16w", name="idx16w")
    for b in range(8):
        nc.sync.dma_start(out=idx16w[16 * b : 16 * (b + 1), :], in_=idx16s[:])

    quad2 = qp.tile([128, W * QELEM], F16, tag="quad2", name="quad2")
    if os.environ.get("QGATHER", "1") == "0":
        nc.vector.memset(quad2[:], 0.0)
    else:
        # chunked: one gather per GX x-columns (idx i = x*128 + p), capped
        # by the scratch-ring limit of dynamic_dma_scratch_size/16 indices
        for x0 in range(0, W, GX):
            xn = min(GX, W - x0)
            _dma_gather_raw(
                nc,
                out_ap=quad2[:, x0 * QELEM : (x0 + xn) * QELEM],
                in_ap=AP(
                    imgcat2,
                    n * CELLROW * CELL_ELEMS,
                    [[CELL_ELEMS, CELLROW], [1, QELEM]],
                ),
                idxs_ap=idx16w[:, x0 * 8 : (x0 + xn) * 8],
                num_idxs=128 * xn,
                elem_size=QELEM,
                queue=(queue + x0 // GX) % NSQ,
                num_idxs_reg=nreg[xn],
            )

    res = qp.tile([128, C * W], F32, tag="res", name="res")
    acc1, acc2 = t("acc1"), t("acc2")
    for c in range(C):
        first = True
        for dx in range(3):
            for r in range(2):
                s = dx * 6 + r * 3 + c
                dst = acc1 if first else acc2
                nc.vector.tensor_tensor(
                    out=dst[:], in0=quad2[:, s :: QELEM], in1=wc[(dx, r)][:], op=A.mult
                )
                if not first:
                    nc.vector.tensor_tensor(out=acc1[:], in0=acc1[:], in1=acc2[:], op=A.add)
                first = False
        nc.vector.tensor_copy(out=res[:, c * W : (c + 1) * W], in_=acc1[:])
    nc.sync.dma_start(
        out=AP(out, n * FAN1 + r0 * W, [[W, rn], [HW, C], [1, W]]),
        in_=res[:rn, :],
    )


_CACHE = {}


def _insert_library_loads(nc):
    """Raw-Bass equivalent of Bacc.insert_library_loads +
    codegen_inst_isa_subclasses: inserts ModifyPoolConfig LOAD_LIB before
    library-tracked Pool instructions (InstDMAGatherAnt lives in `mlp`).
    Without this the gather opcode is undispatchable on the Q7 cores and
    the device wedges (NRT_EXEC_UNIT_UNRECOVERABLE)."""
    import bass_rust as _bass_rust
    from concourse.library_config import all_libraries, standard

    mask = {}
    for lib in all_libraries:
        for t in lib.instructions:
            mask[t] = mask.get(t, 0) | (1 << lib.index)
    _bass_rust.insert_library_loads(nc, mask, len(all_libraries), standard.index)
    mybir.codegen_inst_isa_subclasses(nc)


def kernel(**inputs):
    _patch_tile_drain()
    tmpdir = inputs.pop("_trace_tmpdir", None)
    img = np.asarray(inputs["img"], dtype=np.float32)
    if "nc" not in _CACHE:
        nc = bass.Bass(
            "TRN2", target_bir_lowering=False, debug=False, num_devices=8,
            # the SWDGE gather ring accepts at most 1024 indices per
            # DMAGatherAnt (ucode-fixed; raising dynamic_dma_scratch_size
            # does NOT raise it) — exceeding it wedges the device.
            dynamic_dma_scratch_size=int(os.environ.get("DDS", "16384")),
            num_swdge_queues=NSQ,
        )
        _build(nc)
        _insert_library_loads(nc)
        _CACHE["nc"] = nc
    nc = _CACHE["nc"]

    xs = (2.0 * np.arange(W, dtype=np.float32) + 1.0) / W - 1.0
    ys = (2.0 * np.arange(H, dtype=np.float32) + 1.0) / H - 1.0
    ys_col = np.zeros((2, 128), np.float32)
    ys_col[0] = ys[:128]
    ys_col[1, :96] = ys[128:]
    bias6 = np.array([0, 0, 111.5, 0, 0, 111.5], np.float32).reshape(6, 1)
    common = {
        "W1": np.asarray(inputs["W1"], np.float32),
        "b1c": np.asarray(inputs["b1"], np.float32).reshape(NU1, 1),
        "W2": np.asarray(inputs["W2"], np.float32),
        "b2c": np.asarray(inputs["b2"], np.float32).reshape(6, 1),
        "bias6": bias6,
        "ys_col": ys_col,
        "xs_row": xs,
        "ys_tile": ys[np.arange(NT * 128) % H].astype(np.float32),
        "sel16": np.equal.outer(
            np.arange(128) % 16, np.arange(16)
        ).astype(np.float32),
        "gmask": np.equal.outer(
            np.arange(128) // 16, np.arange(8)
        ).astype(np.float32),
        "nhw_tile": ((np.arange(NT * 128) // H) * HW).astype(np.float32),
    }
    in_maps = [dict(common, img=img[c * NPC : (c + 1) * NPC]) for c in range(8)]
    res = run_bass_kernel_spmd(
        nc, in_maps, core_ids=list(range(8)), tmpdir=tmpdir
    )
    _CACHE["last_results"] = res
    return np.concatenate([res.results[c]["out"] for c in range(8)], axis=0)

